# revision 11
# baseline (speedup 1.0000x reference)
"""Trainium2 Bass kernel for ChannelProjector2D: out[b,h,w,o] = x[b,h,w,c] @ W[c,o].

Strategy (data-parallel over 8 NeuronCores, bf16 I/O to halve HBM traffic):
  - x: [8, 224, 224, 256] f32. Host casts to bf16 and pre-transposes each
    batch image to xt[p, a, j] = x[j, a*128+p]  ([128, 2, 50176] per core),
    so Cin sits on SBUF partitions and the device does zero transposes.
    W [256, 256] is cast to bf16 and pre-arranged [p, a, o] = W[a*128+p, o].
  - Per core: stream row-groups through SBUF. For each 512-row block the PE
    runs 4 matmuls (2 Cout chunks x 2 Cin chunks, W chunk stationary
    [128,128], x moving N=512, bf16 = 1 cycle/row) accumulating
    out^T[o, j] in PSUM f32; ACT/DVE copy PSUM -> SBUF bf16; DMA out
    o-major [128, 2, M]. Host transposes back and upcasts to f32.
  - HBM traffic 25.7 MB in + 25.7 MB out per core (vs 102.8 MB in f32),
    DMA-bound at ~390 GB/s aggregate per core. bf16 quantization of x/W/out
    adds ~2e-3 norm rel err (tolerance 2e-2).
"""

import numpy as np
import ml_dtypes

BF16 = ml_dtypes.bfloat16

P = 128
CIN = 256
COUT = 256
B, H, Wdim = 8, 224, 224
M_CORE = H * Wdim          # 50176 rows per core (one batch image)
N_CORES = 8
GROUP = 3584               # rows per group (1.75 MB bf16 per direction)
NBLK = 512                 # moving-dim block (max moving free size)

_compiled = {}


def build(
    m_core=M_CORE,
    group=GROUP,
    nblk=NBLK,
    xin_bufs=4,
    osb_bufs=8,
    psum_bufs=8,
    split_io=1,
    in_engines=("sync",),
    out_engines=("scalar", "gpsimd"),
):
    import concourse.bass as bass
    import concourse.mybir as mybir
    import concourse.tile as tile
    from concourse import bacc

    f32 = mybir.dt.float32
    bf16 = mybir.dt.bfloat16
    ngroups = m_core // group
    blocks = group // nblk
    assert m_core % group == 0 and group % nblk == 0

    nc = bacc.Bacc(
        "TRN2",
        target_bir_lowering=False,
        debug=False,
        num_devices=N_CORES,
    )
    x_d = nc.declare_dram_parameter("xt", [P, 2, m_core], bf16, isOutput=False)
    w_d = nc.declare_dram_parameter("Wp", [P, 2, COUT], bf16, isOutput=False)
    o_d = nc.declare_dram_parameter("out", [P, 2, m_core], bf16, isOutput=True)

    eng = {"sync": nc.sync, "scalar": nc.scalar, "vector": nc.vector,
           "gpsimd": nc.gpsimd}
    in_qs = [eng[e] for e in in_engines]
    out_qs = [eng[e] for e in out_engines]
    def cp_vector(out, in_):
        nc.vector.tensor_copy(out=out, in_=in_)

    def cp_scalar(out, in_):
        nc.scalar.copy(out=out, in_=in_)

    cp_engines = [cp_vector, cp_scalar]

    with tile.TileContext(nc) as tc:
        with (
            tc.tile_pool(name="const", bufs=1) as cpool,
            tc.tile_pool(name="xin", bufs=xin_bufs) as xpool,
            tc.tile_pool(name="osb", bufs=osb_bufs) as opool,
            tc.tile_pool(name="ps", bufs=psum_bufs, space=bass.MemorySpace.PSUM) as pspool,
        ):
            # w_sb[p, a, o] = W[a*128 + p, o]  (Cin on partitions, 2 chunks)
            w_sb = cpool.tile([P, 2, COUT], bf16)
            nc.sync.dma_start(out=w_sb[:], in_=w_d[:])
            sio = group // split_io
            for g in range(ngroups):
                g0 = g * group
                x_sb = xpool.tile([P, 2, group], bf16)
                for h in range(split_io):
                    nc_slice = slice(h * sio, (h + 1) * sio)
                    in_qs[(g * split_io + h) % len(in_qs)].dma_start(
                        out=x_sb[:, :, nc_slice],
                        in_=x_d[:, :, g0 + h * sio : g0 + (h + 1) * sio],
                    )
                o_sb = opool.tile([P, 2, group], bf16)
                for blk in range(blocks):
                    j0 = blk * nblk
                    for oc in range(2):
                        ps = pspool.tile([P, nblk], f32)
                        for a in range(2):
                            nc.tensor.matmul(
                                ps[:],
                                w_sb[:, a, oc * P : (oc + 1) * P],
                                x_sb[:, a, j0 : j0 + nblk],
                                start=(a == 0),
                                stop=(a == 1),
                            )
                        cp_engines[(blk * 2 + oc) % 2](
                            o_sb[:, oc, j0 : j0 + nblk], ps[:]
                        )
                for h in range(split_io):
                    out_qs[(g * split_io + h) % len(out_qs)].dma_start(
                        out=o_d[:, :, g0 + h * sio : g0 + (h + 1) * sio],
                        in_=o_sb[:, :, h * sio : (h + 1) * sio],
                    )
    nc.compile()
    return nc


def _get_compiled(key="full", **kwargs):
    if key not in _compiled:
        _compiled[key] = build(**kwargs)
    return _compiled[key]


def _prep_inputs(x, W):
    """Returns (xt_shards [8, 128, 2, M] bf16, Wp [128, 2, 256] bf16)."""
    xb = np.ascontiguousarray(x, dtype=np.float32).reshape(N_CORES, M_CORE, CIN)
    xb = xb.astype(BF16)
    xt = np.empty((N_CORES, P, 2, M_CORE), dtype=BF16)
    for i in range(N_CORES):
        # xt[p, a, j] = x[j, a*128+p]
        np.copyto(xt[i], xb[i].reshape(M_CORE, 2, P).transpose(2, 1, 0))
    Wp = np.ascontiguousarray(
        np.asarray(W, dtype=np.float32).astype(BF16).reshape(2, P, COUT).transpose(1, 0, 2)
    )
    return xt, Wp


def _post_output(outs):
    """outs: [8, 128, 2, M] bf16 (o-major) -> [8, 224, 224, 256] f32."""
    res = np.empty((N_CORES, M_CORE, COUT), dtype=np.float32)
    for i in range(N_CORES):
        # out[j, oc*128+p] = outs[i][p, oc, j]
        np.copyto(res[i].reshape(M_CORE, 2, P), outs[i].transpose(2, 1, 0))
    return res.reshape(B, H, Wdim, COUT)


def run_spmd(nc, xt, Wp, trace=False, **kwargs):
    """xt: [n_cores, 128, 2, M] bf16. Returns (stacked raw outs, results obj)."""
    from concourse.bass_utils import run_bass_kernel_spmd

    n = xt.shape[0]
    in_maps = [{"xt": xt[i], "Wp": Wp} for i in range(n)]
    res = run_bass_kernel_spmd(
        nc, in_maps, core_ids=list(range(n)), trace=trace, **kwargs
    )
    outs = np.stack([res.results[i]["out"] for i in range(n)])
    return outs, res


def kernel(x, W):
    xt, Wp = _prep_inputs(x, W)
    nc = _get_compiled("full")
    outs, _ = run_spmd(nc, xt, Wp)
    return _post_output(outs)
